# revision 7
# baseline (speedup 1.0000x reference)
"""AlignmentContrastiveLoss on 8 TRN2 NeuronCores (Bass/Tile, SPMD).

scores[b,c] = sum_j max_i (im[b,1+i,:] . s[c,1+j,:]) over valid i<im_len[b]-1,
j<s_len[c]-3 (max clamped at 0 whenever b has any invalid i), followed by a
diagonal-margin contrastive loss over the [B,B] score matrix.

Strategy:
  - Host: slice+mask, permute the batch (loss is invariant under a joint
    b/c permutation), snake-deal images to 8 cores sorted by length, pack
    valid image regions into per-core slot columns (bf16), pack valid
    sentence words globally (bf16, replicated to all cores), build
    word->sentence indicator blocks (f32).
  - Device: stationary = 128-word S blocks, moving = packed im columns;
    bf16 matmuls accumulate over D in PSUM; DVE segmented max over slots,
    clamp, f32 indicator matmul accumulates scores_T[c, b_local]; one
    AllGather of [128,16] score blocks; every core then computes the full
    margin loss redundantly and writes the same scalar.
"""

import numpy as np

import concourse.bass as bass
import concourse.bacc as bacc
import concourse.tile as tile
import concourse.mybir as mybir
from concourse import bass_utils

try:
    from ml_dtypes import bfloat16
except ImportError:  # jax ships ml_dtypes
    from jax.numpy import bfloat16

N_CORES = 8
MARGIN = 0.2
NEG_BIG = -3.0e38


def _prepare(im_set, s_seq, im_len, s_len):
    """Host-side shard/pack. Returns (meta, in_maps)."""
    im_set = np.ascontiguousarray(np.asarray(im_set, dtype=np.float32))
    s_seq = np.ascontiguousarray(np.asarray(s_seq, dtype=np.float32))
    im_l = np.asarray(im_len).astype(np.int64) - 1
    s_l = np.asarray(s_len).astype(np.int64) - 3

    B = im_set.shape[0]
    D = im_set.shape[2]
    Li = im_set.shape[1] - 1
    Ls = s_seq.shape[1] - 3
    R = B // N_CORES

    im = im_set[:, 1:, :]
    s = s_seq[:, 1 : 1 + Ls, :]
    im_l = np.clip(im_l, 0, Li)
    s_l = np.clip(s_l, 0, Ls)

    # --- permute batch: sort by im_l desc, snake-deal to cores ---
    order = np.argsort(-im_l, kind="stable")
    assign = [[] for _ in range(N_CORES)]
    for idx, b in enumerate(order):
        rnd, pos = divmod(idx, N_CORES)
        core = pos if rnd % 2 == 0 else N_CORES - 1 - pos
        assign[core].append(int(b))
    sigma = np.array([b for m in range(N_CORES) for b in assign[m]])

    # --- slot widths (shared across cores; round to mult of 4, cap Li) ---
    imls = np.array(
        [[im_l[assign[m][r]] for r in range(R)] for m in range(N_CORES)]
    )  # [cores, R]
    wmax = imls.max(axis=0)
    slot_w = np.minimum(((wmax + 3) // 4) * 4, Li).astype(np.int64)
    slot_w = np.maximum(slot_w, 4)
    offs = np.concatenate([[0], np.cumsum(slot_w)]).astype(np.int64)
    n_im = int(offs[-1])

    # split slots into two PSUM tiles at the boundary nearest n_im/2
    split = int(np.argmin(np.abs(offs[1:-1] - n_im / 2))) + 1
    n_a = int(offs[split])

    # runs of equal slot width inside each tile -> one DVE reduce per run
    def runs(lo, hi):
        out = []
        r = lo
        while r < hi:
            r2 = r
            while r2 < hi and slot_w[r2] == slot_w[r]:
                r2 += 1
            out.append((r, r2 - r, int(slot_w[r])))  # (slot_start, n, width)
            r = r2
        return out

    runs_a, runs_b = runs(0, split), runs(split, R)

    # --- per-core moving operand [128 (D part), 8 (D chunk), n_im] bf16 ---
    imt_cores = []
    clamp_cores = []
    for m in range(N_CORES):
        imt = np.zeros((D, n_im), np.float32)
        for r in range(R):
            b = assign[m][r]
            L = int(im_l[b])
            imt[:, offs[r] : offs[r] + L] = im[b, :L, :].T
        imt = imt.astype(bfloat16).reshape(8, 128, n_im).transpose(1, 0, 2)
        imt_cores.append(np.ascontiguousarray(imt))
        clamp = np.where(imls[m] < Li, 0.0, NEG_BIG).astype(np.float32)
        clamp_cores.append(np.ascontiguousarray(np.broadcast_to(clamp, (128, R))))

    # --- packed sentence words, sigma order ---
    n_words = int(s_l.sum())
    G = (n_words + 127) // 128
    w_pad = G * 128
    s_pack = np.zeros((w_pad, D), np.float32)
    word_c = np.full(w_pad, -1, np.int64)
    w = 0
    for p in range(B):
        c_old = sigma[p]
        L = int(s_l[c_old])
        s_pack[w : w + L] = s[c_old, :L, :]
        word_c[w : w + L] = p
        w += L

    # stationary blocks, DMA-friendly: [G, 128 (D part), 8 (D chunk), 128 (word)]
    sb = s_pack.astype(bfloat16).reshape(G, 128, 8, 128)  # [g, w, k, kp]
    s_blocks = np.ascontiguousarray(sb.transpose(0, 3, 2, 1))  # [g, kp, k, w]

    # indicator [G, 128 (word part), 128 (c)] f32
    ind = np.zeros((G, 128, B), np.float32)
    gs, ws = np.divmod(np.arange(w_pad), 128)
    valid = word_c >= 0
    ind[gs[valid], ws[valid], word_c[valid]] = 1.0

    eye = np.ascontiguousarray(np.eye(B, dtype=np.float32))

    meta = dict(B=B, D=D, R=R, n_im=n_im, n_a=n_a, G=G, runs_a=runs_a,
                runs_b=runs_b, offs=offs)
    in_maps = []
    for m in range(N_CORES):
        in_maps.append(
            {
                "imt": imt_cores[m],
                "clamp": clamp_cores[m],
                "s_blocks": s_blocks,
                "ind": ind,
                "eye": eye,
            }
        )
    return meta, in_maps


def _build(meta):
    B, R, n_im, n_a, G = meta["B"], meta["R"], meta["n_im"], meta["n_a"], meta["G"]
    n_b = n_im - n_a
    runs_a, runs_b, offs = meta["runs_a"], meta["runs_b"], meta["offs"]
    f32, bf16 = mybir.dt.float32, mybir.dt.bfloat16

    nc = bacc.Bacc("TRN2", target_bir_lowering=False, debug=False,
                   num_devices=N_CORES)
    imt_d = nc.dram_tensor("imt", [128, 8, n_im], bf16, kind="ExternalInput")
    clamp_d = nc.dram_tensor("clamp", [128, R], f32, kind="ExternalInput")
    s_d = nc.dram_tensor("s_blocks", [G, 128, 8, 128], bf16, kind="ExternalInput")
    ind_d = nc.dram_tensor("ind", [G, 128, B], f32, kind="ExternalInput")
    eye_d = nc.dram_tensor("eye", [B, B], f32, kind="ExternalInput")
    out_d = nc.dram_tensor("out", [1, 1], f32, kind="ExternalOutput")
    dbg_d = nc.dram_tensor("dbg", [128, B], f32, kind="ExternalOutput")

    with tile.TileContext(nc) as tc:
        with (
            tc.tile_pool(name="resident", bufs=1) as resident,
            tc.tile_pool(name="sblk", bufs=4) as sblk_pool,
            tc.tile_pool(name="indp", bufs=4) as ind_pool,
            tc.tile_pool(name="maxv", bufs=4) as maxv_pool,
            tc.tile_pool(name="psA", bufs=2, space="PSUM") as psA_pool,
            tc.tile_pool(name="psB", bufs=2, space="PSUM") as psB_pool,
            tc.tile_pool(name="psS", bufs=1, space="PSUM") as psS_pool,
            tc.tile_pool(name="tail", bufs=1) as tailp,
            tc.tile_pool(name="dram", bufs=1, space="DRAM") as dram,
        ):
            # resident tiles
            imt_sb = resident.tile([128, 8, n_im], bf16)
            nc.sync.dma_start(out=imt_sb[:], in_=imt_d[:])
            clamp_sb = resident.tile([128, R], f32)
            nc.sync.dma_start(out=clamp_sb[:], in_=clamp_d[:])
            eye_sb = resident.tile([B, B], f32)
            nc.sync.dma_start(out=eye_sb[:], in_=eye_d[:])
            ones_sb = resident.tile([B, B], f32)
            nc.gpsimd.memset(ones_sb[:], 1.0)
            ones_col = resident.tile([128, 1], f32)
            nc.gpsimd.memset(ones_col[:], 1.0)

            scores_ps = psS_pool.tile([128, R], f32)

            for g in range(G):
                s_sb = sblk_pool.tile([128, 8, 128], bf16)
                nc.sync.dma_start(out=s_sb[:], in_=s_d[g])
                ind_sb = ind_pool.tile([128, B], f32)
                nc.sync.dma_start(out=ind_sb[:], in_=ind_d[g])

                psA = psA_pool.tile([128, n_a], f32)
                psB = psB_pool.tile([128, n_b], f32)
                for k in range(8):
                    nc.tensor.matmul(
                        psA[:], s_sb[:, k, :], imt_sb[:, k, 0:n_a],
                        start=(k == 0), stop=(k == 7),
                    )
                    nc.tensor.matmul(
                        psB[:], s_sb[:, k, :], imt_sb[:, k, n_a:n_im],
                        start=(k == 0), stop=(k == 7),
                    )

                maxv = maxv_pool.tile([128, R], f32)
                for r0, nr, wdt in runs_a:
                    src = psA[:, offs[r0] : offs[r0] + nr * wdt]
                    nc.vector.tensor_reduce(
                        maxv[:, r0 : r0 + nr],
                        src.rearrange("p (n w) -> p n w", w=wdt),
                        axis=mybir.AxisListType.X,
                        op=mybir.AluOpType.max,
                    )
                for r0, nr, wdt in runs_b:
                    base = offs[r0] - n_a
                    src = psB[:, base : base + nr * wdt]
                    nc.vector.tensor_reduce(
                        maxv[:, r0 : r0 + nr],
                        src.rearrange("p (n w) -> p n w", w=wdt),
                        axis=mybir.AxisListType.X,
                        op=mybir.AluOpType.max,
                    )
                nc.vector.tensor_tensor(
                    maxv[:], maxv[:], clamp_sb[:], op=mybir.AluOpType.max
                )
                nc.tensor.matmul(
                    scores_ps[:], ind_sb[:], maxv[:],
                    start=(g == 0), stop=(g == G - 1),
                )

            # ---- tail: AllGather + full loss on every core ----
            scores_sb = tailp.tile([128, R], f32)
            nc.vector.tensor_copy(scores_sb[:], scores_ps[:])
            cc_in = dram.tile([128, R], f32)
            nc.sync.dma_start(out=cc_in[:], in_=scores_sb[:])
            cc_out = dram.tile([N_CORES, 128, R], f32)
            nc.gpsimd.collective_compute(
                "AllGather",
                mybir.AluOpType.bypass,
                replica_groups=[list(range(N_CORES))],
                ins=[cc_in[:].opt()],
                outs=[cc_out[:].opt()],
            )
            t_sb = tailp.tile([128, B], f32)  # T[c, b]
            for m in range(N_CORES):
                nc.sync.dma_start(
                    out=t_sb[:, m * R : (m + 1) * R], in_=cc_out[m]
                )

            nc.sync.dma_start(out=dbg_d[:], in_=t_sb[:])
            masked = tailp.tile([128, B], f32)
            nc.vector.tensor_tensor(
                masked[:], t_sb[:], eye_sb[:], op=mybir.AluOpType.mult
            )
            diag_col = tailp.tile([128, 1], f32)
            nc.vector.tensor_reduce(
                diag_col[:], masked[:], axis=mybir.AxisListType.X,
                op=mybir.AluOpType.add,
            )
            # mneg = MARGIN - diag
            mneg = tailp.tile([128, 1], f32)
            nc.vector.tensor_scalar(
                mneg[:], diag_col[:], -1.0, MARGIN,
                op0=mybir.AluOpType.mult, op1=mybir.AluOpType.add,
            )
            # cost_im = relu(T + (margin - diag[c])), sum along free dim
            sum_im = tailp.tile([128, 1], f32)
            tmp_im = tailp.tile([128, B], f32)
            nc.vector.tensor_scalar(
                tmp_im[:], t_sb[:], mneg[:, 0:1], 0.0,
                op0=mybir.AluOpType.add, op1=mybir.AluOpType.max,
            )
            nc.vector.tensor_reduce(
                sum_im[:], tmp_im[:], axis=mybir.AxisListType.X,
                op=mybir.AluOpType.add,
            )
            # Gmat[c,b] = diag[b] via ones^T @ masked
            gmat_ps = psS_pool.tile([128, B], f32)
            nc.tensor.matmul(gmat_ps[:], ones_sb[:], masked[:], start=True, stop=True)
            tmp_s = tailp.tile([128, B], f32)
            nc.vector.tensor_tensor(
                tmp_s[:], t_sb[:], gmat_ps[:], op=mybir.AluOpType.subtract
            )
            sum_s = tailp.tile([128, 1], f32)
            tmp_s2 = tailp.tile([128, B], f32)
            nc.vector.tensor_scalar(
                tmp_s2[:], tmp_s[:], MARGIN, 0.0,
                op0=mybir.AluOpType.add, op1=mybir.AluOpType.max,
            )
            nc.vector.tensor_reduce(
                sum_s[:], tmp_s2[:], axis=mybir.AxisListType.X,
                op=mybir.AluOpType.add,
            )
            tot = tailp.tile([128, 1], f32)
            nc.vector.tensor_tensor(
                tot[:], sum_im[:], sum_s[:], op=mybir.AluOpType.add
            )
            # partition sum via matmul: [1,1] = tot^T @ ones_col
            fin_ps = psS_pool.tile([1, 1], f32)
            nc.tensor.matmul(fin_ps[:], tot[:], ones_col[:], start=True, stop=True)
            res_sb = tailp.tile([1, 1], f32)
            # subtract the diagonal contribution 2*B*MARGIN
            nc.vector.tensor_scalar(
                res_sb[:], fin_ps[:], -2.0 * B * MARGIN, None,
                op0=mybir.AluOpType.add,
            )
            nc.sync.dma_start(out=out_d[:], in_=res_sb[:])

    nc.compile()
    return nc


def run(im_set, s_seq, im_len, s_len, trace=False):
    meta, in_maps = _prepare(im_set, s_seq, im_len, s_len)
    nc = _build(meta)
    res = bass_utils.run_bass_kernel_spmd(
        nc, in_maps, core_ids=list(range(N_CORES)), trace=trace
    )
    val = np.float32(res.results[0]["out"][0, 0])
    return np.asarray(val, dtype=np.float32).reshape(()), res


def kernel(im_set, s_seq, im_len, s_len):
    out, _ = run(im_set, s_seq, im_len, s_len, trace=False)
    return out


# revision 9
# speedup vs baseline: 1.2539x; 1.2539x over previous
"""AlignmentContrastiveLoss on 8 TRN2 NeuronCores (Bass/Tile, SPMD).

scores[b,c] = sum_j max_i (im[b,1+i,:] . s[c,1+j,:]) over valid i<im_len[b]-1,
j<s_len[c]-3 (the max also includes 0 whenever b has any invalid i), followed
by a diagonal-margin contrastive loss over the [B,B] score matrix.

Strategy:
  - Host: slice, permute the batch (loss is invariant under a joint b/c
    permutation), snake-deal images to 8 cores sorted by length, pack valid
    image regions into per-core slot columns (bf16; every slot holding a
    short image keeps >=1 zero pad column so the reduce reproduces the
    reference max-with-0), pack valid sentence words globally (bf16,
    replicated), word->sentence indicator blocks (bf16).
  - Device: stationary = 128-word S blocks, moving = packed im columns;
    bf16 matmuls accumulate over D in PSUM; DVE segmented max over slot
    width classes; bf16 indicator matmul accumulates scores_T[c, b_local];
    one AllGather of [128,16] score blocks; every core then computes the
    full margin loss redundantly and writes the same scalar.
"""

import numpy as np

import concourse.bass as bass
import concourse.bacc as bacc
import concourse.tile as tile
import concourse.mybir as mybir
from concourse import bass_utils

try:
    from ml_dtypes import bfloat16
except ImportError:  # jax ships ml_dtypes
    from jax.numpy import bfloat16

N_CORES = 8
MARGIN = 0.2


def _choose_classes(widths):
    """Partition sorted-desc slot widths into classes (round width up to the
    class value). DP minimizing: per-run DVE overhead ~125ns + per-pad-col
    ~4.4ns (PE stream + DVE element)."""
    ws = sorted(widths, reverse=True)
    n = len(ws)
    RUN = 125.0
    PAD = 4.4
    # dp[i] = min cost covering ws[i:]
    INF = float("inf")
    dp = [INF] * (n + 1)
    dp[n] = 0.0
    choice = [0] * n
    for i in range(n - 1, -1, -1):
        w = ws[i]
        for j in range(i + 1, n + 1):
            pad = sum(w - ws[k] for k in range(i, j))
            c = RUN + PAD * pad + dp[j]
            if c < dp[i]:
                dp[i] = c
                choice[i] = j
    out = []
    i = 0
    while i < n:
        j = choice[i]
        out.append((i, j - i, ws[i]))  # (slot_start, count, width)
        i = j
    return out


def _prepare(im_set, s_seq, im_len, s_len):
    """Host-side shard/pack. Returns (meta, in_maps)."""
    im_set = np.ascontiguousarray(np.asarray(im_set, dtype=np.float32))
    s_seq = np.ascontiguousarray(np.asarray(s_seq, dtype=np.float32))
    im_l = np.asarray(im_len).astype(np.int64) - 1
    s_l = np.asarray(s_len).astype(np.int64) - 3

    B = im_set.shape[0]
    D = im_set.shape[2]
    Li = im_set.shape[1] - 1
    Ls = s_seq.shape[1] - 3
    R = B // N_CORES

    im = im_set[:, 1:, :]
    s = s_seq[:, 1 : 1 + Ls, :]
    im_l = np.clip(im_l, 0, Li)
    s_l = np.clip(s_l, 0, Ls)

    # --- permute batch: sort by im_l desc, snake-deal to cores ---
    order = np.argsort(-im_l, kind="stable")
    assign = [[] for _ in range(N_CORES)]
    for idx, b in enumerate(order):
        rnd, pos = divmod(idx, N_CORES)
        core = pos if rnd % 2 == 0 else N_CORES - 1 - pos
        assign[core].append(int(b))
    sigma = np.array([b for m in range(N_CORES) for b in assign[m]])

    # --- slot widths (shared across cores) ---
    # effective width forces >=1 zero pad for short images so the reduce's
    # max includes 0 exactly as the reference's zero-masked tail does
    imls = np.array(
        [[im_l[assign[m][r]] for r in range(R)] for m in range(N_CORES)]
    )  # [cores, R]
    eff = np.minimum(imls + (imls < Li), Li)
    wmax = eff.max(axis=0)  # [R], non-increasing by construction of assign
    runs = _choose_classes(list(wmax))
    slot_w = np.zeros(R, np.int64)
    for r0, nr, wdt in runs:
        slot_w[r0 : r0 + nr] = wdt
    assert np.all(slot_w >= wmax)
    offs = np.concatenate([[0], np.cumsum(slot_w)]).astype(np.int64)
    n_im = int(offs[-1])

    # segments of slots with cumulative width <= 512 (PSUM bank limit)
    segs = []  # (slot_lo, slot_hi, col_lo, col_hi)
    lo = 0
    for r in range(R + 1):
        if r == R or offs[r + 1] - offs[lo] > 512:
            segs.append((lo, r, int(offs[lo]), int(offs[r])))
            lo = r
    assert segs[-1][1] == R

    # --- per-core moving operand [128 (D part), 8 (D chunk), n_im] bf16 ---
    imt_cores = []
    for m in range(N_CORES):
        imt = np.zeros((D, n_im), np.float32)
        for r in range(R):
            b = assign[m][r]
            L = int(im_l[b])
            imt[:, offs[r] : offs[r] + L] = im[b, :L, :].T
        imt = imt.astype(bfloat16).reshape(8, 128, n_im).transpose(1, 0, 2)
        imt_cores.append(np.ascontiguousarray(imt))

    # --- packed sentence words, sigma order ---
    n_words = int(s_l.sum())
    G = (n_words + 127) // 128
    w_pad = G * 128
    s_pack = np.zeros((w_pad, D), np.float32)
    word_c = np.full(w_pad, -1, np.int64)
    w = 0
    for p in range(B):
        c_old = sigma[p]
        L = int(s_l[c_old])
        s_pack[w : w + L] = s[c_old, :L, :]
        word_c[w : w + L] = p
        w += L

    # merged per-block stream: [G, 128 (part), 8*128 (s chunks) + B (ind)] bf16
    sb = s_pack.astype(bfloat16).reshape(G, 128, 8, 128)  # [g, w, k, kp]
    s_blk = sb.transpose(0, 3, 2, 1)  # [g, kp, k, w]
    ind = np.zeros((G, 128, B), bfloat16)
    gs, ws_ = np.divmod(np.arange(w_pad), 128)
    valid = word_c >= 0
    ind[gs[valid], ws_[valid], word_c[valid]] = 1.0
    blk = np.concatenate(
        [s_blk.reshape(G, 128, 8 * 128), ind], axis=2
    )  # [G, 128, 1024+B]
    blk = np.ascontiguousarray(blk)

    eye = np.ascontiguousarray(np.eye(B, dtype=np.float32))

    meta = dict(B=B, D=D, R=R, n_im=n_im, G=G, runs=runs, segs=segs, offs=offs)
    in_maps = []
    for m in range(N_CORES):
        in_maps.append({"imt": imt_cores[m], "blk": blk, "eye": eye})
    return meta, in_maps


def _build(meta):
    B, R, n_im, G = meta["B"], meta["R"], meta["n_im"], meta["G"]
    runs, segs, offs = meta["runs"], meta["segs"], meta["offs"]
    f32, bf16 = mybir.dt.float32, mybir.dt.bfloat16
    KC = meta["D"] // 128  # contraction chunks

    nc = bacc.Bacc("TRN2", target_bir_lowering=False, debug=False,
                   num_devices=N_CORES)
    imt_d = nc.dram_tensor("imt", [128, KC, n_im], bf16, kind="ExternalInput")
    blk_d = nc.dram_tensor("blk", [G, 128, KC * 128 + B], bf16,
                           kind="ExternalInput")
    eye_d = nc.dram_tensor("eye", [B, B], f32, kind="ExternalInput")
    out_d = nc.dram_tensor("out", [1, 1], f32, kind="ExternalOutput")
    dbg_d = nc.dram_tensor("dbg", [128, B], f32, kind="ExternalOutput")

    # runs per segment (split any run that crosses a segment boundary)
    seg_runs = []
    for (slo, shi, clo, chi) in segs:
        rr = []
        for r0, nr, wdt in runs:
            lo, hi = max(r0, slo), min(r0 + nr, shi)
            if lo < hi:
                rr.append((lo, hi - lo, wdt))
        seg_runs.append(rr)

    with tile.TileContext(nc) as tc:
        with (
            tc.tile_pool(name="resident", bufs=1) as resident,
            tc.tile_pool(name="blkp", bufs=4) as blk_pool,
            tc.tile_pool(name="maxv", bufs=4) as maxv_pool,
            tc.tile_pool(name="ps", bufs=2 + (len(segs) == 1), space="PSUM")
            as ps_pool,
            tc.tile_pool(name="psS", bufs=1, space="PSUM") as psS_pool,
            tc.tile_pool(name="tail", bufs=1) as tailp,
            tc.tile_pool(name="dram", bufs=1, space="DRAM") as dram,
        ):
            # resident tiles
            imt_sb = resident.tile([128, KC, n_im], bf16)
            nc.scalar.dma_start(out=imt_sb[:], in_=imt_d[:])
            eye_sb = resident.tile([B, B], f32)
            nc.scalar.dma_start(out=eye_sb[:], in_=eye_d[:])
            ones_sb = resident.tile([B, B], f32)
            nc.gpsimd.memset(ones_sb[:], 1.0)
            ones_col = resident.tile([128, 1], f32)
            nc.gpsimd.memset(ones_col[:], 1.0)

            scores_ps = psS_pool.tile([128, R], f32)

            for g in range(G):
                blk_sb = blk_pool.tile([128, KC * 128 + B], bf16)
                nc.sync.dma_start(out=blk_sb[:], in_=blk_d[g])

                ps_tiles = []
                for si, (slo, shi, clo, chi) in enumerate(segs):
                    ps = ps_pool.tile([128, chi - clo], f32, tag=f"ps{si}")
                    ps_tiles.append(ps)
                for k in range(KC):
                    for si, (slo, shi, clo, chi) in enumerate(segs):
                        nc.tensor.matmul(
                            ps_tiles[si][:],
                            blk_sb[:, k * 128 : (k + 1) * 128],
                            imt_sb[:, k, clo:chi],
                            start=(k == 0),
                            stop=(k == KC - 1),
                        )

                maxv = maxv_pool.tile([128, R], bf16)
                for si, (slo, shi, clo, chi) in enumerate(segs):
                    for r0, nr, wdt in seg_runs[si]:
                        base = int(offs[r0]) - clo
                        src = ps_tiles[si][:, base : base + nr * wdt]
                        nc.vector.tensor_reduce(
                            maxv[:, r0 : r0 + nr],
                            src.rearrange("p (n w) -> p n w", w=wdt),
                            axis=mybir.AxisListType.X,
                            op=mybir.AluOpType.max,
                        )
                nc.tensor.matmul(
                    scores_ps[:],
                    blk_sb[:, KC * 128 : KC * 128 + B],
                    maxv[:],
                    start=(g == 0),
                    stop=(g == G - 1),
                )

            # ---- tail: AllGather + full loss on every core ----
            scores_sb = tailp.tile([128, R], f32)
            nc.vector.tensor_copy(scores_sb[:], scores_ps[:])
            cc_in = dram.tile([128, R], f32)
            nc.sync.dma_start(out=cc_in[:], in_=scores_sb[:])
            cc_out = dram.tile([N_CORES, 128, R], f32)
            nc.gpsimd.collective_compute(
                "AllGather",
                mybir.AluOpType.bypass,
                replica_groups=[list(range(N_CORES))],
                ins=[cc_in[:].opt()],
                outs=[cc_out[:].opt()],
            )
            t_sb = tailp.tile([128, B], f32)  # T[c, b]
            for m in range(N_CORES):
                nc.sync.dma_start(
                    out=t_sb[:, m * R : (m + 1) * R], in_=cc_out[m]
                )
            nc.sync.dma_start(out=dbg_d[:], in_=t_sb[:])

            masked = tailp.tile([128, B], f32)
            nc.vector.tensor_tensor(
                masked[:], t_sb[:], eye_sb[:], op=mybir.AluOpType.mult
            )
            diag_col = tailp.tile([128, 1], f32)
            nc.vector.tensor_reduce(
                diag_col[:], masked[:], axis=mybir.AxisListType.X,
                op=mybir.AluOpType.add,
            )
            # mneg = MARGIN - diag
            mneg = tailp.tile([128, 1], f32)
            nc.vector.tensor_scalar(
                mneg[:], diag_col[:], -1.0, MARGIN,
                op0=mybir.AluOpType.mult, op1=mybir.AluOpType.add,
            )
            # cost_im = relu(T + (margin - diag[c]))
            sum_im = tailp.tile([128, 1], f32)
            tmp_im = tailp.tile([128, B], f32)
            nc.vector.tensor_scalar(
                tmp_im[:], t_sb[:], mneg[:, 0:1], 0.0,
                op0=mybir.AluOpType.add, op1=mybir.AluOpType.max,
            )
            nc.vector.tensor_reduce(
                sum_im[:], tmp_im[:], axis=mybir.AxisListType.X,
                op=mybir.AluOpType.add,
            )
            # Gmat[c,b] = diag[b] via ones^T @ masked
            gmat_ps = psS_pool.tile([128, B], f32)
            nc.tensor.matmul(gmat_ps[:], ones_sb[:], masked[:], start=True,
                             stop=True)
            tmp_s = tailp.tile([128, B], f32)
            nc.vector.tensor_tensor(
                tmp_s[:], t_sb[:], gmat_ps[:], op=mybir.AluOpType.subtract
            )
            sum_s = tailp.tile([128, 1], f32)
            tmp_s2 = tailp.tile([128, B], f32)
            nc.vector.tensor_scalar(
                tmp_s2[:], tmp_s[:], MARGIN, 0.0,
                op0=mybir.AluOpType.add, op1=mybir.AluOpType.max,
            )
            nc.vector.tensor_reduce(
                sum_s[:], tmp_s2[:], axis=mybir.AxisListType.X,
                op=mybir.AluOpType.add,
            )
            tot = tailp.tile([128, 1], f32)
            nc.vector.tensor_tensor(
                tot[:], sum_im[:], sum_s[:], op=mybir.AluOpType.add
            )
            # partition sum via matmul: [1,1] = tot^T @ ones_col
            fin_ps = psS_pool.tile([1, 1], f32)
            nc.tensor.matmul(fin_ps[:], tot[:], ones_col[:], start=True,
                             stop=True)
            res_sb = tailp.tile([1, 1], f32)
            # subtract the diagonal contribution 2*B*MARGIN
            nc.vector.tensor_scalar(
                res_sb[:], fin_ps[:], -2.0 * B * MARGIN, None,
                op0=mybir.AluOpType.add,
            )
            nc.sync.dma_start(out=out_d[:], in_=res_sb[:])

    nc.compile()
    return nc


def run(im_set, s_seq, im_len, s_len, trace=False):
    meta, in_maps = _prepare(im_set, s_seq, im_len, s_len)
    nc = _build(meta)
    res = bass_utils.run_bass_kernel_spmd(
        nc, in_maps, core_ids=list(range(N_CORES)), trace=trace
    )
    val = np.float32(res.results[0]["out"][0, 0])
    return np.asarray(val, dtype=np.float32).reshape(()), res


def kernel(im_set, s_seq, im_len, s_len):
    out, _ = run(im_set, s_seq, im_len, s_len, trace=False)
    return out


# revision 15
# speedup vs baseline: 1.2553x; 1.0011x over previous
"""AlignmentContrastiveLoss on 8 TRN2 NeuronCores (Bass/Tile, SPMD).

scores[b,c] = sum_j max_i (im[b,1+i,:] . s[c,1+j,:]) over valid i<im_len[b]-1,
j<s_len[c]-3 (the max also includes 0 whenever b has any invalid i), followed
by a diagonal-margin contrastive loss over the [B,B] score matrix.

Strategy:
  - Host: slice, permute the batch (loss is invariant under a joint b/c
    permutation), snake-deal images to 8 cores sorted by length, pack valid
    image regions into per-core slot columns (bf16; every slot holding a
    short image keeps >=1 zero pad column so the reduce reproduces the
    reference max-with-0), pack valid sentence words globally (bf16,
    replicated), word->sentence indicator blocks (bf16).
  - Device: stationary = 128-word S blocks, moving = packed im columns;
    bf16 matmuls accumulate over D in PSUM; DVE segmented max over slot
    width classes; bf16 indicator matmul accumulates scores_T[c, b_local];
    one AllGather of [128,16] score blocks; every core then computes the
    full margin loss redundantly and writes the same scalar.
"""

import numpy as np

import concourse.bass as bass
import concourse.bacc as bacc
import concourse.tile as tile
import concourse.mybir as mybir
from concourse import bass_utils

try:
    from ml_dtypes import bfloat16
except ImportError:  # jax ships ml_dtypes
    from jax.numpy import bfloat16

N_CORES = 8
MARGIN = 0.2


def _choose_classes(widths):
    """Partition sorted-desc slot widths into classes (round width up to the
    class value). DP minimizing: per-run DVE overhead ~125ns + per-pad-col
    ~4.4ns (PE stream + DVE element)."""
    ws = sorted(widths, reverse=True)
    n = len(ws)
    RUN = 125.0
    PAD = 4.4
    # dp[i] = min cost covering ws[i:]
    INF = float("inf")
    dp = [INF] * (n + 1)
    dp[n] = 0.0
    choice = [0] * n
    for i in range(n - 1, -1, -1):
        w = ws[i]
        for j in range(i + 1, n + 1):
            pad = sum(w - ws[k] for k in range(i, j))
            c = RUN + PAD * pad + dp[j]
            if c < dp[i]:
                dp[i] = c
                choice[i] = j
    out = []
    i = 0
    while i < n:
        j = choice[i]
        out.append((i, j - i, ws[i]))  # (slot_start, count, width)
        i = j
    return out


def _prepare(im_set, s_seq, im_len, s_len):
    """Host-side shard/pack. Returns (meta, in_maps)."""
    im_set = np.ascontiguousarray(np.asarray(im_set, dtype=np.float32))
    s_seq = np.ascontiguousarray(np.asarray(s_seq, dtype=np.float32))
    im_l = np.asarray(im_len).astype(np.int64) - 1
    s_l = np.asarray(s_len).astype(np.int64) - 3

    B = im_set.shape[0]
    D = im_set.shape[2]
    Li = im_set.shape[1] - 1
    Ls = s_seq.shape[1] - 3
    R = B // N_CORES

    im = im_set[:, 1:, :]
    s = s_seq[:, 1 : 1 + Ls, :]
    im_l = np.clip(im_l, 0, Li)
    s_l = np.clip(s_l, 0, Ls)

    # --- permute batch: sort by im_l desc, snake-deal to cores ---
    order = np.argsort(-im_l, kind="stable")
    assign = [[] for _ in range(N_CORES)]
    for idx, b in enumerate(order):
        rnd, pos = divmod(idx, N_CORES)
        core = pos if rnd % 2 == 0 else N_CORES - 1 - pos
        assign[core].append(int(b))
    sigma = np.array([b for m in range(N_CORES) for b in assign[m]])

    # --- slot widths (shared across cores) ---
    # effective width forces >=1 zero pad for short images so the reduce's
    # max includes 0 exactly as the reference's zero-masked tail does
    imls = np.array(
        [[im_l[assign[m][r]] for r in range(R)] for m in range(N_CORES)]
    )  # [cores, R]
    eff = np.minimum(imls + (imls < Li), Li)
    wmax = eff.max(axis=0)  # [R], non-increasing by construction of assign
    runs = _choose_classes(list(wmax))
    slot_w = np.zeros(R, np.int64)
    for r0, nr, wdt in runs:
        slot_w[r0 : r0 + nr] = wdt
    assert np.all(slot_w >= wmax)
    offs = np.concatenate([[0], np.cumsum(slot_w)]).astype(np.int64)
    n_im = int(offs[-1])

    # segments of slots with cumulative width <= 512 (PSUM bank limit)
    segs = []  # (slot_lo, slot_hi, col_lo, col_hi)
    lo = 0
    for r in range(R + 1):
        if r == R or offs[r + 1] - offs[lo] > 512:
            segs.append((lo, r, int(offs[lo]), int(offs[r])))
            lo = r
    assert segs[-1][1] == R

    # --- per-core moving operand [128 (D part), 8 (D chunk), n_im] bf16 ---
    imt_cores = []
    for m in range(N_CORES):
        imt = np.zeros((D, n_im), np.float32)
        for r in range(R):
            b = assign[m][r]
            L = int(im_l[b])
            imt[:, offs[r] : offs[r] + L] = im[b, :L, :].T
        imt = imt.astype(bfloat16).reshape(8, 128, n_im).transpose(1, 0, 2)
        imt_cores.append(np.ascontiguousarray(imt))

    # --- packed sentence words, sigma order ---
    n_words = int(s_l.sum())
    G = (n_words + 127) // 128
    w_pad = G * 128
    s_pack = np.zeros((w_pad, D), np.float32)
    word_c = np.full(w_pad, -1, np.int64)
    w = 0
    for p in range(B):
        c_old = sigma[p]
        L = int(s_l[c_old])
        s_pack[w : w + L] = s[c_old, :L, :]
        word_c[w : w + L] = p
        w += L

    # merged per-block stream: [G, 128 (part), 8*128 (s chunks) + B (ind)] bf16
    sb = s_pack.astype(bfloat16).reshape(G, 128, 8, 128)  # [g, w, k, kp]
    s_blk = sb.transpose(0, 3, 2, 1)  # [g, kp, k, w]
    ind = np.zeros((G, 128, B), bfloat16)
    gs, ws_ = np.divmod(np.arange(w_pad), 128)
    valid = word_c >= 0
    ind[gs[valid], ws_[valid], word_c[valid]] = 1.0
    blk = np.concatenate(
        [s_blk.reshape(G, 128, 8 * 128), ind], axis=2
    )  # [G, 128, 1024+B]
    blk = np.ascontiguousarray(blk)

    eye = np.ascontiguousarray(np.eye(B, dtype=np.float32))

    meta = dict(B=B, D=D, R=R, n_im=n_im, G=G, runs=runs, segs=segs, offs=offs)
    in_maps = []
    for m in range(N_CORES):
        in_maps.append({"imt": imt_cores[m], "blk": blk, "eye": eye})
    return meta, in_maps


def _build(meta):
    B, R, n_im, G = meta["B"], meta["R"], meta["n_im"], meta["G"]
    runs, segs, offs = meta["runs"], meta["segs"], meta["offs"]
    f32, bf16 = mybir.dt.float32, mybir.dt.bfloat16
    KC = meta["D"] // 128  # contraction chunks

    nc = bacc.Bacc("TRN2", target_bir_lowering=False, debug=False,
                   num_devices=N_CORES)
    imt_d = nc.dram_tensor("imt", [128, KC, n_im], bf16, kind="ExternalInput")
    blk_d = nc.dram_tensor("blk", [G, 128, KC * 128 + B], bf16,
                           kind="ExternalInput")
    eye_d = nc.dram_tensor("eye", [B, B], f32, kind="ExternalInput")
    out_d = nc.dram_tensor("out", [1, 1], f32, kind="ExternalOutput")
    dbg_d = nc.dram_tensor("dbg", [128, B], f32, kind="ExternalOutput")

    # runs per segment (split any run that crosses a segment boundary)
    seg_runs = []
    for (slo, shi, clo, chi) in segs:
        rr = []
        for r0, nr, wdt in runs:
            lo, hi = max(r0, slo), min(r0 + nr, shi)
            if lo < hi:
                rr.append((lo, hi - lo, wdt))
        seg_runs.append(rr)

    with tile.TileContext(nc) as tc:
        with (
            tc.tile_pool(name="resident", bufs=1) as resident,
            tc.tile_pool(name="blkp", bufs=5) as blk_pool,
            tc.tile_pool(name="maxv", bufs=4) as maxv_pool,
            tc.tile_pool(
                name="ps",
                bufs=max(2, min(4, 5 // len(segs))),
                space="PSUM",
            ) as ps_pool,
            tc.tile_pool(name="psS", bufs=1, space="PSUM") as psS_pool,
            tc.tile_pool(name="tail", bufs=1) as tailp,
            tc.tile_pool(name="dram", bufs=1, space="DRAM") as dram,
        ):
            # resident tiles
            imt_sb = resident.tile([128, KC, n_im], bf16)
            for k in range(KC):
                nc.scalar.dma_start(out=imt_sb[:, k, :], in_=imt_d[:, k, :])
            eye_sb = resident.tile([B, B], f32)
            nc.scalar.dma_start(out=eye_sb[:], in_=eye_d[:])
            ones_sb = resident.tile([B, B], bf16)
            nc.gpsimd.memset(ones_sb[:], 1.0)
            ones_col = resident.tile([128, 1], bf16)
            nc.gpsimd.memset(ones_col[:], 1.0)

            scores_ps = psS_pool.tile([128, R], f32)

            for g in range(G):
                blk_sb = blk_pool.tile([128, KC * 128 + B], bf16)
                nc.sync.dma_start(out=blk_sb[:], in_=blk_d[g])

                ps_tiles = []
                for si, (slo, shi, clo, chi) in enumerate(segs):
                    ps = ps_pool.tile([128, chi - clo], f32, tag=f"ps{si}")
                    ps_tiles.append(ps)
                for k in range(KC):
                    for si, (slo, shi, clo, chi) in enumerate(segs):
                        nc.tensor.matmul(
                            ps_tiles[si][:],
                            blk_sb[:, k * 128 : (k + 1) * 128],
                            imt_sb[:, k, clo:chi],
                            start=(k == 0),
                            stop=(k == KC - 1),
                        )

                maxv = maxv_pool.tile([128, R], bf16)
                for si, (slo, shi, clo, chi) in enumerate(segs):
                    for r0, nr, wdt in seg_runs[si]:
                        base = int(offs[r0]) - clo
                        src = ps_tiles[si][:, base : base + nr * wdt]
                        nc.vector.tensor_reduce(
                            maxv[:, r0 : r0 + nr],
                            src.rearrange("p (n w) -> p n w", w=wdt),
                            axis=mybir.AxisListType.X,
                            op=mybir.AluOpType.max,
                        )
                nc.tensor.matmul(
                    scores_ps[:],
                    blk_sb[:, KC * 128 : KC * 128 + B],
                    maxv[:],
                    start=(g == 0),
                    stop=(g == G - 1),
                )

            # ---- tail: AllGather + full loss on every core ----
            scores_sb = tailp.tile([128, R], f32)
            nc.vector.tensor_copy(scores_sb[:], scores_ps[:])
            cc_in = dram.tile([128, R], f32)
            nc.sync.dma_start(out=cc_in[:], in_=scores_sb[:])
            cc_out = dram.tile([N_CORES, 128, R], f32)
            nc.gpsimd.collective_compute(
                "AllGather",
                mybir.AluOpType.bypass,
                replica_groups=[list(range(N_CORES))],
                ins=[cc_in[:].opt()],
                outs=[cc_out[:].opt()],
            )
            t_sb = tailp.tile([128, B], f32)  # T[c, b]
            nc.sync.dma_start(
                out=t_sb[:].rearrange("p (m r) -> p m r", m=N_CORES),
                in_=cc_out[:].rearrange("m c r -> c m r"),
            )
            nc.sync.dma_start(out=dbg_d[:], in_=t_sb[:])

            masked = tailp.tile([128, B], f32)
            nc.vector.tensor_tensor(
                masked[:], t_sb[:], eye_sb[:], op=mybir.AluOpType.mult
            )
            masked_bf = tailp.tile([128, B], bf16)
            nc.vector.tensor_tensor(
                masked_bf[:], t_sb[:], eye_sb[:], op=mybir.AluOpType.mult
            )
            diag_col = tailp.tile([128, 1], f32)
            nc.vector.tensor_reduce(
                diag_col[:], masked[:], axis=mybir.AxisListType.X,
                op=mybir.AluOpType.add,
            )
            # mneg = MARGIN - diag
            mneg = tailp.tile([128, 1], f32)
            nc.vector.tensor_scalar(
                mneg[:], diag_col[:], -1.0, MARGIN,
                op0=mybir.AluOpType.mult, op1=mybir.AluOpType.add,
            )
            # cost_im = relu(T + (margin - diag[c]))
            sum_im = tailp.tile([128, 1], f32)
            tmp_im = tailp.tile([128, B], f32)
            nc.vector.tensor_scalar(
                tmp_im[:], t_sb[:], mneg[:, 0:1], 0.0,
                op0=mybir.AluOpType.add, op1=mybir.AluOpType.max,
            )
            nc.vector.tensor_reduce(
                sum_im[:], tmp_im[:], axis=mybir.AxisListType.X,
                op=mybir.AluOpType.add,
            )
            # Gmat[c,b] = diag[b] via ones^T @ masked
            gmat_ps = psS_pool.tile([128, B], f32)
            nc.tensor.matmul(gmat_ps[:], ones_sb[:], masked_bf[:], start=True,
                             stop=True)
            tmp_s = tailp.tile([128, B], f32)
            nc.vector.tensor_tensor(
                tmp_s[:], t_sb[:], gmat_ps[:], op=mybir.AluOpType.subtract
            )
            sum_s = tailp.tile([128, 1], f32)
            tmp_s2 = tailp.tile([128, B], f32)
            nc.vector.tensor_scalar(
                tmp_s2[:], tmp_s[:], MARGIN, 0.0,
                op0=mybir.AluOpType.add, op1=mybir.AluOpType.max,
            )
            nc.vector.tensor_reduce(
                sum_s[:], tmp_s2[:], axis=mybir.AxisListType.X,
                op=mybir.AluOpType.add,
            )
            tot = tailp.tile([128, 1], bf16)
            nc.vector.tensor_tensor(
                tot[:], sum_im[:], sum_s[:], op=mybir.AluOpType.add
            )
            # partition sum via matmul: [1,1] = tot^T @ ones_col
            fin_ps = psS_pool.tile([1, 1], f32)
            nc.tensor.matmul(fin_ps[:], tot[:], ones_col[:], start=True,
                             stop=True)
            res_sb = tailp.tile([1, 1], f32)
            # subtract the diagonal contribution 2*B*MARGIN
            nc.vector.tensor_scalar(
                res_sb[:], fin_ps[:], -2.0 * B * MARGIN, None,
                op0=mybir.AluOpType.add,
            )
            nc.sync.dma_start(out=out_d[:], in_=res_sb[:])

    nc.compile()
    return nc


def run(im_set, s_seq, im_len, s_len, trace=False):
    meta, in_maps = _prepare(im_set, s_seq, im_len, s_len)
    nc = _build(meta)
    res = bass_utils.run_bass_kernel_spmd(
        nc, in_maps, core_ids=list(range(N_CORES)), trace=trace
    )
    val = np.float32(res.results[0]["out"][0, 0])
    return np.asarray(val, dtype=np.float32).reshape(()), res


def kernel(im_set, s_seq, im_len, s_len):
    out, _ = run(im_set, s_seq, im_len, s_len, trace=False)
    return out
